# revision 1
# baseline (speedup 1.0000x reference)
"""Low-rank multi-head attention Bass kernel for Trainium2 (8 NeuronCores).

Sharding: (batch, query-block) data parallel. 8 cores = 2 batches x 4 query
blocks. Each core receives only its own query block of x, computes k1/v1 for
that block, and AllGathers them within each 4-core batch group. Keys are
consumed in per-core rolled order (own block first; softmax is invariant to
key order), which lets local-block attention run while the collective is in
flight. The rolled placement uses partition_id-dependent dynamic DMA source
offsets.

Structure:
  * Wv2h and Wo1_h fold into a per-head 32x32 matrix Mh = Wv2h @ Wo1_h.T, so
    attention applies to Z = v1 @ Mh (32+ones cols per head) instead of Vh,
    and the o1 projection disappears: o1 = sum_h softmax(scores_h) @ Zh_aug.
  * x is loaded with a casting gpsimd DMA (f32->bf16) and transposed by the
    DMA XBAR -- the PE does no x transposes.
  * All hot matmuls run in bf16 (1 col/cycle, no small-tile penalty).
  * exp() alternates per head between ACT (exact, bf16 out) and DVE
    (Schraudolph bit-trick: i16 = round(s*a + b) bitcast as bf16).
  * Per-head score tiles (one PSUM bank each, ring of 4) keep both exp
    engines streaming back-to-back.
  * attn@Z accumulates into 60 persistent PSUM slots ([queries, 34] per
    (head, query-chunk)); softmax division happens in a batched epilogue,
    followed by a per-query-chunk out-projection pipeline.
"""

import os
import sys

sys.path.insert(0, "/opt/trn_rl_repo")

from contextlib import ExitStack

import numpy as np

import concourse.bass as bass
import concourse.tile as tile
from concourse import bacc
from concourse import mybir
from concourse.masks import make_identity

F32 = mybir.dt.float32
F32R = mybir.dt.float32r
BF16 = mybir.dt.bfloat16
I16 = mybir.dt.int16
AF = mybir.ActivationFunctionType
ALU = mybir.AluOpType

H, D, R, N = 20, 64, 32, 1280

# feature flags (bisectable)
USE_XBAR = True       # DMA XBAR x-transpose (else PE f32 transposes)
USE_CAST_DMA = True   # gpsimd casting DMAs for x / weights
USE_DYN_DMA = True    # partition_id-based rolled k/v placement
USE_SCHRAUDOLPH = True  # DVE bit-trick exp for odd heads
KSTAGE = int(os.environ.get("KSTAGE", "9"))  # debug stage bisect
NCORES = 8
QP = 4  # query blocks per batch
SCALE = float(D) ** -0.5  # 0.125

# Schraudolph exp -> bf16 bits: i16 = round(x * 128/ln2 + (127*128 - C))
SCH_A = SCALE * 128.0 / float(np.log(2.0))
SCH_B = 127.0 * 128.0 - 0.0579848 * 128.0


def _chunks(total, size, start=0):
    out = []
    s = start
    while s < total:
        out.append((s, min(size, total - s)))
        s += size
    return out


def build_nc(S, SQ):
    nc = bacc.Bacc("TRN2", target_bir_lowering=False, debug=False, num_devices=NCORES)

    xb = nc.dram_tensor("xb", [SQ, N], F32, kind="ExternalInput")
    Wq1 = nc.dram_tensor("Wq1", [R, N], F32, kind="ExternalInput")
    Wq2 = nc.dram_tensor("Wq2", [N, R], F32, kind="ExternalInput")
    bq = nc.dram_tensor("bq", [N], F32, kind="ExternalInput")
    Wk1 = nc.dram_tensor("Wk1", [R, N], F32, kind="ExternalInput")
    Wk2 = nc.dram_tensor("Wk2", [N, R], F32, kind="ExternalInput")
    bk = nc.dram_tensor("bk", [N], F32, kind="ExternalInput")
    Wv1 = nc.dram_tensor("Wv1", [R, N], F32, kind="ExternalInput")
    Wv2 = nc.dram_tensor("Wv2", [N, R], F32, kind="ExternalInput")
    bv = nc.dram_tensor("bv", [N], F32, kind="ExternalInput")
    Wo1 = nc.dram_tensor("Wo1", [R, N], F32, kind="ExternalInput")
    Wo2 = nc.dram_tensor("Wo2", [N, R], F32, kind="ExternalInput")
    bo = nc.dram_tensor("bo", [N], F32, kind="ExternalInput")
    out = nc.dram_tensor("out", [SQ, N], F32, kind="ExternalOutput")

    SQP = SQ + (SQ % 2)                      # 376: even moving dim
    ICH = _chunks(SQ, 128)                   # query chunks (3)
    # key chunks: 3 per 375-row block (blocks never straddled); local first
    SCH = [(SQ * m + o, w) for m in range(QP) for (o, w) in _chunks(SQ, 128)]
    NJ = len(SCH)
    NIC = len(ICH)
    OSUB = _chunks(N, 512)
    NC10 = N // 128

    def mm(out_, lhsT, rhs, **kw):
        nc.tensor.matmul(out_, lhsT, rhs, **kw)

    ev = [0]

    def evac(dst, src):
        # round-robin psum evacuations between DVE and ACT
        ev[0] += 1
        if ev[0] % 2 == 0:
            nc.scalar.copy(dst, src)
        else:
            nc.vector.tensor_copy(dst, src)

    def exp_on_dve(J, h):
        return USE_SCHRAUDOLPH and h % 2 == 1

    with tile.TileContext(nc) as tc, ExitStack() as ctx:
        wp = ctx.enter_context(tc.tile_pool(name="wp", bufs=1))
        small_p = ctx.enter_context(tc.tile_pool(name="small_p", bufs=4))
        at2_p = ctx.enter_context(tc.tile_pool(name="at2_p", bufs=12))
        outp = ctx.enter_context(tc.tile_pool(name="outp", bufs=2))
        psA = ExitStack()
        pse = psA.enter_context(tc.tile_pool(name="pse", bufs=2, space="PSUM"))

        # ---- persistent SBUF tensors ----
        ident = wp.tile([128, 128], F32)
        make_identity(nc, ident[:])

        WkvT = wp.tile([128, 640], BF16)     # [Wk1.T | Wv1.T] interleaved per chunk
        WqT = wp.tile([128, 320], BF16)
        WoT = wp.tile([128, 320], BF16)      # Wo1.T chunks
        Wv2c = wp.tile([64, 640], BF16)      # Wv2 head-major: [d, 32h+r]
        WoT2 = wp.tile([64, 640], BF16)      # Wo1.T head-major: [d, 32h+s]
        Wq2aug = wp.tile([64, 660], F32)     # [Wq2 | bq] head-major, d on partitions
        Wk2aug = wp.tile([64, 660], F32)
        bk_c = wp.tile([64, 20], F32)
        bv_c = wp.tile([128, 12], BF16)
        bo_row = wp.tile([1, N], F32)
        Wo2Ta = wp.tile([33, N], BF16)       # rows 0:32 Wo2.T, row 32 = Wo2@Wo1@bv + bo
        t1sb = wp.tile([32, 2], BF16)
        wsb2 = wp.tile([128, 320], F32)

        k1b = [
            wp.tile([33, SQP], BF16, name=f"k1b{m}", tag=f"k1b{m}") for m in range(QP)
        ]  # per rolled block: rows 0:32 k1T, row 32 ones
        v1b = [
            wp.tile([32, SQP], BF16, name=f"v1b{m}", tag=f"v1b{m}") for m in range(QP)
        ]
        Mall = wp.tile([32, 640], BF16)      # Mh side by side
        q1Ta = wp.tile([33, SQP], BF16)      # rows 0:32 q1T, row 32 ones; col SQ zero
        o1Ta = wp.tile([33, SQP], BF16)
        Zt = [wp.tile([128, 680], BF16, name=f"Zt{j}", tag=f"Zt{j}") for j in range(NJ)]
        # Zt layout: 20 heads x [32 z-cols | 1.0 | 0.0]; the ones col feeds
        # the softmax denominator through the same accumulating matmul.
        qh_all = [
            wp.tile([33, SQP], BF16, name=f"qh{h}", tag=f"qh{h}") for h in range(H)
        ]
        Yg = [wp.tile([128, 480], F32, name=f"Yg{t}", tag=f"Yg{t}") for t in range(4)]
        rrt = wp.tile([128, 64], F32)        # reciprocals, 16 per group
        o1grp = wp.tile([128, 96], F32)      # [i, ic, r] final o1 (pre-transpose)
        scr = wp.tile([128, 576], F32)       # per-ic epilogue scratch (3 x 192)

        def build_z(j, pool, tag, bufs=None):
            j0, p = SCH[j]
            mp_, off_ = j0 // SQ, j0 % SQ
            for half in range(2):
                zp = pool.tile([128, 320], F32, tag=tag, name="zp", bufs=bufs)
                mm(
                    zp[:p, :],
                    v1b[mp_][:, off_ : off_ + p],
                    Mall[:, 320 * half : 320 * half + 320],
                )
                zdst = Zt[j][:p, 340 * half : 340 * half + 340].rearrange(
                    "p (h c) -> p h c", c=34
                )[:, :, 0:32]
                zsrc = zp[:p, :].rearrange("p (h c) -> p h c", c=32)
                if (half + j) % 2 == 0:
                    nc.scalar.copy(zdst, zsrc)
                else:
                    nc.vector.tensor_copy(zdst, zsrc)

        # ================= setup: x, k/v, collective =================
        with tc.tile_pool(name="xin_p", bufs=3) as xin_p, tc.tile_pool(
            name="wload", bufs=3
        ) as wload, tc.tile_pool(name="xT_p", bufs=1) as xT_p:
            # x loads: casting gpsimd DMAs (f32->bf16 in flight, halves the
            # bytes on the shared DMA pipe)
            xbf = []
            xf32 = []
            for ic, (i0, iw) in enumerate(ICH):
                xt_ = xin_p.tile([128, N], BF16, tag="xin", name=f"xbf{ic}")
                if USE_CAST_DMA:
                    nc.gpsimd.dma_start(xt_[:iw, :], xb[i0 : i0 + iw, :])
                else:
                    xf = xin_p.tile([128, N], F32, tag="xf32", name=f"xf{ic}")
                    nc.sync.dma_start(xf[:iw, :], xb[i0 : i0 + iw, :])
                    xf32.append(xf)
                xbf.append(xt_)

            # small weight loads, issued after x (ready immediately, consumed
            # later than the XBAR transposes that follow them in the FIFOs)
            wsb_k = wload.tile([32, N], F32, tag="wsb_k")
            nc.sync.dma_start(wsb_k[:], Wk1[:])
            wsb_v = wload.tile([32, N], F32, tag="wsb_v")
            nc.scalar.dma_start(wsb_v[:], Wv1[:])
            wsb_q = wload.tile([32, N], F32, tag="wsb_q")
            wsb_o = wload.tile([32, N], F32, tag="wsb_o")

            # PE p-state warm-up
            warm_ps = pse.tile([128, 320], F32, tag="ps")
            for _ in range(18):
                mm(warm_ps[:, 0:128], ident[:], ident[:], is_transpose=True)

            # xTall: feature f%128 on partitions; block c at cols 384c;
            # query q of chunk ic at col 384c + 128*ic + (q - 128*ic)
            xTall = xT_p.tile([128, 384 * NC10], BF16)
            if USE_XBAR:
                for ic in range(NIC):
                    eng = nc.sync if ic % 2 == 0 else nc.scalar
                    dst = xTall[:, :].rearrange("p (c w) -> p c w", w=384)[
                        :, :, 128 * ic : 128 * ic + 128
                    ]
                    eng.dma_start_transpose(dst, xbf[ic][0:128, :])
            else:
                for c in range(NC10):
                    tpx = pse.tile([128, 512], F32, tag="ps")
                    for ic, (i0, iw) in enumerate(ICH):
                        mm(
                            tpx[:, 128 * ic : 128 * ic + iw],
                            xf32[ic][:iw, 128 * c : 128 * c + 128],
                            ident[:iw, :iw],
                            is_transpose=True,
                        )
                    evac(
                        xTall[:, 384 * c : 384 * c + SQ],
                        tpx[:, 0:SQ],
                    )

            # --- Wk1/Wv1 -> WkvT (f32 transposes, bf16 evacs) ---
            for wi, wsb in enumerate((wsb_k, wsb_v)):
                tp = pse.tile([128, 320], F32, tag="ps")
                for c in range(NC10):
                    mm(
                        tp[:, 32 * c : 32 * c + 32],
                        wsb[:, 128 * c : 128 * c + 128],
                        ident[:32, :32],
                        is_transpose=True,
                    )
                dst = WkvT[:, :].rearrange("p (c two r) -> p c two r", two=2, r=32)[
                    :, :, wi
                ]
                evac(dst, tp[:, :].rearrange("p (c r) -> p c r", r=32))

            # --- local k1/v1 ([64, SQP] psum: k rows 0:32, v rows 32:64) ---
            kv_ps = pse.tile([64, SQP], F32, tag="ps")
            for c in range(NC10):
                mm(
                    kv_ps[:],
                    WkvT[:, 64 * c : 64 * c + 64],
                    xTall[:, 384 * c : 384 * c + SQP],
                    start=(c == 0),
                    stop=(c == NC10 - 1),
                )
            if USE_DYN_DMA:
                nc.vector.tensor_copy(k1b[0][0:32, 0:SQ], kv_ps[0:32, 0:SQ])
                nc.scalar.copy(v1b[0][:, 0:SQ], kv_ps[32:64, 0:SQ])
            else:
                kv_sb = wload.tile([64, SQP], BF16, tag="kv_sb")
                nc.vector.tensor_copy(kv_sb[:, 0:SQ], kv_ps[:, 0:SQ])

            # --- collective: gather local k1/v1 within the batch group ---
            with tc.tile_pool(name="dramp", bufs=1, space="DRAM") as dramp:
                cc_in = dramp.tile([64, SQ], BF16)
                cc_out = dramp.tile([64 * QP, SQ], BF16)
                if USE_DYN_DMA:
                    nc.sync.dma_start(cc_in[0:32, :], k1b[0][0:32, 0:SQ])
                    nc.scalar.dma_start(cc_in[32:64, :], v1b[0][:, 0:SQ])
                else:
                    nc.sync.dma_start(cc_in[:, :], kv_sb[:, 0:SQ])
                nc.gpsimd.collective_compute(
                    "AllGather",
                    ALU.bypass,
                    replica_groups=[
                        list(range(g * QP, (g + 1) * QP)) for g in range(NCORES // QP)
                    ],
                    ins=[cc_in[:].opt()],
                    outs=[cc_out[:].opt()],
                )

                nc.sync.dma_start(
                    Wq2aug[:, :].rearrange("d (h r) -> d h r", r=33)[:, :, 0:32],
                    Wq2[:].rearrange("(h d) r -> d h r", d=64),
                )
                nc.scalar.dma_start(
                    Wk2aug[:, :].rearrange("d (h r) -> d h r", r=33)[:, :, 0:32],
                    Wk2[:].rearrange("(h d) r -> d h r", d=64),
                )
                nc.sync.dma_start(
                    Wq2aug[:, :].rearrange("d (h r) -> d h r", r=33)[:, :, 32:33],
                    bq[:].rearrange("(h d) -> d h", d=64).unsqueeze(2),
                )
                nc.scalar.dma_start(
                    Wk2aug[:, :].rearrange("d (h r) -> d h r", r=33)[:, :, 32:33],
                    bk[:].rearrange("(h d) -> d h", d=64).unsqueeze(2),
                )
                nc.sync.dma_start(bk_c[:], bk[:].rearrange("(h d) -> d h", d=64))
                if USE_CAST_DMA:
                    nc.gpsimd.dma_start(wsb_q[:], Wq1[:])
                    nc.gpsimd.dma_start(wsb_o[:], Wo1[:])
                else:
                    nc.sync.dma_start(wsb_q[:], Wq1[:])
                    nc.scalar.dma_start(wsb_o[:], Wo1[:])
                for m in range(QP):
                    nc.gpsimd.memset(k1b[m][32:33, :], 1.0)
                for j in range(NJ):
                    z3 = Zt[j][:, :].rearrange("p (h c) -> p h c", c=34)
                    nc.gpsimd.memset(z3[:, :, 32:33], 1.0)
                    nc.gpsimd.memset(z3[:, :, 33:34], 0.0)

                # remote k1/v1: rolled block mp holds gather block (qi+mp)%4
                if USE_DYN_DMA:
                    qi = nc.sync.partition_id() % QP
                    qi2 = nc.scalar.partition_id() % QP
                    for mp in range(1, QP):
                        b = (qi + mp) % QP
                        b2 = (qi2 + mp) % QP
                        nc.sync.dma_start(
                            k1b[mp][0:32, 0:SQ], cc_out[bass.DynSlice(b * 64, 32), :]
                        )
                        nc.scalar.dma_start(
                            v1b[mp][:, 0:SQ],
                            cc_out[bass.DynSlice(b2 * 64 + 32, 32), :],
                        )
                else:
                    # static: gather block m at position m for every core
                    # (keys in global order; own-block overlap is lost)
                    for mp in range(QP):
                        nc.sync.dma_start(
                            k1b[mp][0:32, 0:SQ], cc_out[64 * mp : 64 * mp + 32, :]
                        )
                        nc.scalar.dma_start(
                            v1b[mp][:, 0:SQ], cc_out[64 * mp + 32 : 64 * mp + 64, :]
                        )

                if KSTAGE >= 1:
                    # --- q-side prep (overlaps the collective) ---
                    tp = pse.tile([128, 320], F32, tag="ps")
                    for c in range(NC10):
                        mm(
                            tp[:, 32 * c : 32 * c + 32],
                            wsb_q[:, 128 * c : 128 * c + 128],
                            ident[:32, :32],
                            is_transpose=True,
                        )
                    evac(WqT[:], tp[:])
                    q1ps = pse.tile([32, SQP], F32, tag="ps")
                    for c in range(NC10):
                        mm(
                            q1ps[:],
                            WqT[:, 32 * c : 32 * c + 32],
                            xTall[:, 384 * c : 384 * c + SQP],
                            start=(c == 0),
                            stop=(c == NC10 - 1),
                        )
                    nc.vector.tensor_copy(q1Ta[0:32, :], q1ps[:])
                    nc.gpsimd.memset(q1Ta[32:33, :], 1.0)
                    if SQP > SQ:
                        nc.gpsimd.memset(q1Ta[0:33, SQ:SQP], 0.0)

                    # --- Mall: Mh = Wv2h @ Wo1_h.T ---
                    tp = pse.tile([128, 320], F32, tag="ps")
                    for c in range(NC10):
                        mm(
                            tp[:, 32 * c : 32 * c + 32],
                            wsb_o[:, 128 * c : 128 * c + 128],
                            ident[:32, :32],
                            is_transpose=True,
                        )
                    evac(WoT[:], tp[:])
                    # head-major copy for the Mall matmuls (all partitions 0:64)
                    w2v = WoT2[:, :].rearrange("d (hc two r) -> d hc two r", two=2, r=32)
                    t3v = tp[:, :].rearrange("p (c r) -> p c r", r=32)
                    nc.vector.tensor_copy(w2v[:, :, 0], t3v[0:64, :, :])
                    nc.scalar.copy(w2v[:, :, 1], t3v[64:128, :, :])
                    if USE_CAST_DMA:
                        nc.gpsimd.dma_start(
                            Wv2c[:, :].rearrange("d (h r) -> d h r", r=32),
                            Wv2[:].rearrange("(h d) r -> d h r", d=64),
                        )
                    else:
                        wv2f = wload.tile([64, 640], F32, tag="wv2f")
                        nc.sync.dma_start(
                            wv2f[:, :].rearrange("d (h r) -> d h r", r=32),
                            Wv2[:].rearrange("(h d) r -> d h r", d=64),
                        )
                        nc.vector.tensor_copy(Wv2c[:], wv2f[:])
                    mall_ps = pse.tile([32, 640], F32, tag="big", bufs=2)
                    for h in range(H):
                        mm(
                            mall_ps[:, 32 * h : 32 * h + 32],
                            Wv2c[:, 32 * h : 32 * h + 32],
                            WoT2[:, 32 * h : 32 * h + 32],
                            skip_group_check=True,
                        )
                    evac(Mall[:], mall_ps[:])

                    # --- wm + qh per head ---
                    for h in range(H):
                        wmps = pse.tile([33, 33], F32, tag="ps")
                        mm(
                            wmps[0:32, 0:32],
                            Wq2aug[:, 33 * h : 33 * h + 32],
                            Wk2aug[:, 33 * h : 33 * h + 32],
                        )
                        mm(
                            wmps[32:33, 0:32],
                            Wq2aug[:, 33 * h + 32 : 33 * h + 33],
                            Wk2aug[:, 33 * h : 33 * h + 32],
                            skip_group_check=True,
                        )
                        mm(
                            wmps[0:33, 32:33],
                            Wq2aug[:, 33 * h : 33 * h + 33],
                            bk_c[:, h : h + 1],
                            skip_group_check=True,
                        )
                        wm = small_p.tile([33, 33], BF16, tag="wm_sb")
                        nc.vector.tensor_copy(wm[:], wmps[:])
                        qhps = pse.tile([33, SQP], F32, tag="qh")
                        mm(qhps[:], wm[:], q1Ta[:])
                        evac(qh_all[h][:], qhps[:])


                    # local Z chunks
                    for j in range(NIC):
                        build_z(j, pse, "big", bufs=2)


        # ================= attention =================
        psA.close()
        psB = ExitStack()
        ps_acc = psB.enter_context(tc.tile_pool(name="ps_acc", bufs=1, space="PSUM"))
        psC = ExitStack()
        ps_sc = psC.enter_context(tc.tile_pool(name="ps_sc", bufs=4, space="PSUM"))

        accT = [
            ps_acc.tile([128, 510], F32, name=f"acc{t}", tag=f"acc{t}")
            for t in range(4)
        ]

        for J, (j0, p) in enumerate(SCH if KSTAGE >= 2 else []):
            mp, off = j0 // SQ, j0 % SQ
            if J + 1 < NJ and J + 1 >= NIC:
                build_z(J + 1, ps_sc, "sc")
            for h in range(H):
                sc = ps_sc.tile([128, 512], F32, tag="sc")
                mm(sc[:p, 0:SQP], k1b[mp][:, off : off + p], qh_all[h][:])
                at2 = at2_p.tile([128, SQP], BF16, tag="at")
                if exp_on_dve(J, h):
                    nc.vector.tensor_scalar(
                        at2[:p, :].bitcast(I16),
                        sc[:p, 0:SQP],
                        SCH_A,
                        SCH_B,
                        ALU.mult,
                        ALU.add,
                    )
                else:
                    nc.scalar.activation(at2[:p, :], sc[:p, 0:SQP], AF.Exp, scale=SCALE)
                t, base = h // 5, (h % 5) * 3
                for ic, (i0, iw) in enumerate(ICH):
                    c0 = 34 * (base + ic)
                    # one accumulation group per PSUM bank: start zeroes the
                    # whole bank row in every written partition, so only the
                    # very first matmul into the bank may set it
                    mm(
                        accT[t][0:iw, c0 : c0 + 34],
                        at2[:p, i0 : i0 + iw],
                        Zt[J][:p, 34 * h : 34 * h + 34],
                        start=(J == 0 and h % 5 == 0 and ic == 0),
                        stop=(J == NJ - 1 and h % 5 == 4 and ic == 2),
                        skip_group_check=True,
                    )
                if J == NJ - 1 and h % 5 == 4 and KSTAGE >= 3:
                    # tile h//5 is complete: divide by the softmax denominator
                    # now, while the remaining exps still occupy the engines
                    td = h // 5
                    acc3 = accT[td][:, :].rearrange("p (s c) -> p s c", c=34)
                    nc.vector.reciprocal(
                        rrt[:, 16 * td : 16 * td + 15].unsqueeze(2), acc3[:, :, 32:33]
                    )
                    y_ap = Yg[td][:, :].rearrange("p (s c) -> p s c", c=32)
                    rb, ab = bass.broadcast_tensor_aps(
                        rrt[:, 16 * td : 16 * td + 15].unsqueeze(2), acc3[:, :, 0:32]
                    )
                    nc.vector.tensor_tensor(y_ap, ab, rb, ALU.mult)

        if KSTAGE < 3:
            # debug dumps (gpsimd DMAs cast bf16 -> f32)
            for m in range(QP):
                nc.gpsimd.dma_start(
                    out[33 * m : 33 * m + 33, 0:SQP], k1b[m][:, :]
                )
                nc.gpsimd.dma_start(
                    out[140 + 32 * m : 140 + 32 * m + 32, 0:SQP], v1b[m][:, :]
                )
            if KSTAGE >= 1:
                nc.gpsimd.dma_start(out[280:313, 0:SQP], qh_all[0][:, :])
                nc.gpsimd.dma_start(out[313:346, 0:SQP], qh_all[19][:, :])
                nc.gpsimd.dma_start(out[343:375, 640:1280], Mall[:, :])
                nc.gpsimd.dma_start(out[0:128, 400:1080], Zt[0][:, :])
                nc.gpsimd.dma_start(out[128:256, 400:1080], Zt[NJ - 1][:, :])
                nc.gpsimd.dma_start(out[256:289, 400:776], q1Ta[:, :])
            if KSTAGE == 2:
                for t in range(4):
                    nc.vector.tensor_copy(Yg[t][:, 0:480], accT[t][:, 0:480])
                    nc.sync.dma_start(
                        out[93 * t : 93 * t + 93, 800:1280], Yg[t][0:93, 0:480]
                    )

        # ================= tail: Wo2Ta prep, epilogue, out-projection ======
        psC.close()
        psD = ExitStack()
        psf = psD.enter_context(tc.tile_pool(name="psf", bufs=3, space="PSUM"))

        if KSTAGE >= 3:
            # Wo2 -> Wo2Ta (bf16) + bo_eff into row 32
            nc.gpsimd.memset(bv_c[:], 0.0)
            if USE_CAST_DMA:
                nc.gpsimd.dma_start(
                    bv_c[:, 0:10], bv[:].rearrange("(c p) -> p c", p=128)
                )
            else:
                bvf = outp.tile([128, 10], F32, tag="bvf")
                nc.sync.dma_start(bvf[:], bv[:].rearrange("(c p) -> p c", p=128))
                nc.vector.tensor_copy(bv_c[:, 0:10], bvf[:])
            nc.sync.dma_start(bo_row[:], bo[:].unsqueeze(0))
            nc.sync.dma_start(
                wsb2[:].rearrange("p (c r) -> p c r", r=32),
                Wo2[:].rearrange("(c p) r -> p c r", p=128),
            )
            nc.gpsimd.memset(o1Ta[32:33, :], 1.0)
            if SQP > SQ:
                nc.gpsimd.memset(o1Ta[0:33, SQ:SQP], 0.0)
            for g0 in range(0, NC10, 4):
                gn = min(4, NC10 - g0)
                tp2 = psf.tile([32, 512], F32, tag="fp")
                for k in range(gn):
                    c = g0 + k
                    mm(
                        tp2[:, 128 * k : 128 * k + 128],
                        wsb2[:, 32 * c : 32 * c + 32],
                        ident[:],
                        is_transpose=True,
                    )
                evac(Wo2Ta[0:32, 128 * g0 : 128 * (g0 + gn)], tp2[:, : 128 * gn])
            t1ps = psf.tile([32, 2], F32, tag="fp")
            for c in range(NC10):
                mm(
                    t1ps[:],
                    WoT[:, 32 * c : 32 * c + 32],
                    bv_c[:, c : c + 2],
                    start=(c == 0),
                    stop=(c == NC10 - 1),
                )
            nc.vector.tensor_copy(t1sb[:], t1ps[:])
            for (n0, nw) in OSUB:
                beps = psf.tile([1, 512], F32, tag="fp")
                mm(beps[:, :nw], t1sb[:, 0:1], Wo2Ta[0:32, n0 : n0 + nw])
                nc.vector.tensor_add(
                    Wo2Ta[32:33, n0 : n0 + nw], beps[:, :nw], bo_row[:, n0 : n0 + nw]
                )

            # (softmax division now interleaved into the last chunk)

            # epilogue part 2 + out-projection, pipelined per query chunk
            o1ps = psf.tile([32, SQP], F32, tag="o1t", bufs=1)
            for ic, (i0, iw) in enumerate(ICH):
                # sum the 20 heads: 4 groups x 5 heads, slot = 3*hl + ic
                yi = [
                    Yg[t][:, :].rearrange("p (hl icc c) -> p hl icc c", c=32, icc=3)[
                        :, :, ic
                    ]
                    for t in range(4)
                ]
                sA = scr[:, 192 * ic : 192 * ic + 160].rearrange("p (hl c) -> p hl c", c=32)
                sB = scr[:, 192 * ic + 160 : 192 * ic + 192].rearrange(
                    "p (z c) -> p z c", c=32
                )
                nc.vector.tensor_tensor(sA, yi[0], yi[1], ALU.add)
                nc.gpsimd.tensor_tensor(sA, sA, yi[2], ALU.add)
                nc.vector.tensor_tensor(sA, sA, yi[3], ALU.add)
                nc.vector.tensor_tensor(sA[:, 0:2], sA[:, 0:2], sA[:, 2:4], ALU.add)
                nc.gpsimd.tensor_tensor(sB, sA[:, 0:1], sA[:, 1:2], ALU.add)
                nc.vector.tensor_tensor(
                    o1grp[:, 32 * ic : 32 * ic + 32].unsqueeze(1), sB, sA[:, 4:5], ALU.add
                )
                mm(
                    o1ps[:, i0 : i0 + iw],
                    o1grp[0:iw, 32 * ic : 32 * ic + 32],
                    ident[:iw, :iw],
                    is_transpose=True,
                )
                evac(o1Ta[0:32, i0 : i0 + iw], o1ps[:, i0 : i0 + iw])
                osb = outp.tile([128, N], F32, tag="osb")
                for (n0, nw) in OSUB:
                    fps = psf.tile([128, 512], F32, tag="fp")
                    mm(fps[:iw, :nw], o1Ta[:, i0 : i0 + iw], Wo2Ta[:, n0 : n0 + nw])
                    evac(osb[:iw, n0 : n0 + nw], fps[:iw, :nw])
                (nc.sync if ic % 2 == 0 else nc.gpsimd).dma_start(
                    out[i0 : i0 + iw, :], osb[:iw, :]
                )

        psD.close()
        psB.close()

    nc.compile()
    return nc


_NC_CACHE = {}


def _get_nc(S, SQ):
    key = (S, SQ)
    if key not in _NC_CACHE:
        _NC_CACHE[key] = build_nc(S, SQ)
    return _NC_CACHE[key]


def kernel(**inputs):
    from concourse.bass_utils import run_bass_kernel_spmd

    x = np.asarray(inputs["x"], dtype=np.float32)
    B, S, n = x.shape
    assert n == N and B * QP == NCORES
    SQ = S // QP
    nc = _get_nc(S, SQ)

    wnames = [
        "Wq1", "Wq2", "bq", "Wk1", "Wk2", "bk",
        "Wv1", "Wv2", "bv", "Wo1", "Wo2", "bo",
    ]
    weights = {
        k: np.ascontiguousarray(np.asarray(inputs[k], dtype=np.float32))
        for k in wnames
    }

    in_maps = []
    for core in range(NCORES):
        b, qi = divmod(core, QP)
        m = {"xb": np.ascontiguousarray(x[b, SQ * qi : SQ * (qi + 1)])}
        m.update(weights)
        in_maps.append(m)

    res = run_bass_kernel_spmd(nc, in_maps, core_ids=list(range(NCORES)))
    outs = res.results if hasattr(res, "results") else res

    out = np.zeros((B, S, N), dtype=np.float32)
    for core in range(NCORES):
        b, qi = divmod(core, QP)
        out[b, SQ * qi : SQ * (qi + 1), :] = outs[core]["out"]
    return out

